# revision 16
# baseline (speedup 1.0000x reference)
"""Trainium2 Bass kernel for nn_ClusterMemory (scatter_memory).

Reference computation (B=256, D=2048, S=65536, TEMP=0.05):
    x = inputs / ||inputs||_row            # [B, D]
    logits = (x @ features.T) / TEMP       # [B, S]
    loss = mean_i( logsumexp(logits[i,:]) - logits[i, targets[i]] )

Because both x rows and features rows are L2-normalized, every logit is a
cosine / TEMP, bounded to [-20, 20] -> exp() never overflows in f32 and no
max-subtraction pass is needed.  So each of the 8 cores only has to return
S_shard[i] = sum_j exp(logits[i, j]) over its 8192-row shard of the memory
bank (features sharded row-wise, per the sharding hint).  The softmax
normalizer combine (a [256]-vector sum over 8 shards) and the target-logit
term (256 dot products; targets are row-parallel but trivially cheap) are
done on host in f64.

Device kernel per core (memory-bound: streams its 64 MB feature shard once).
The shard is packed on host into DMA-native tile order
[N_CHUNKS, 128, KT, JC] so every chunk DMA is one fully-contiguous 4 MB
read (32 KB per partition):
    xT      [128, 16, 256]           f32  (normalized inputs, packed)
    featsT  [N_CHUNKS, 128, 16, 512] f32  (feature shard, packed)
    s_out   [128, 2]                 f32  (s_out[p, h] = S for item h*128+p)

    for each of 16 j-chunks (512 memory rows):
        DMA chunk -> SBUF [128, 16, 512]
        for each batch half (2 x 128):
            16 matmuls accumulate PSUM[128b, 512j] over K=2048
            ACT: exp(psum / TEMP) with accum_out -> per-row partial sum
    reduce the 16 partial sums per half, DMA [128, 2] out.

Matmul runs in float32r (PE-native reduced-precision fp32: 1 cyc/row vs 4
for exact fp32; HW-probed max rel err ~1.4e-4 per K=128 dot, which washes
out to ~1e-6 on the final loss).
"""

import os
import numpy as np

import concourse.bacc as bacc
import concourse.bass as bass
import concourse.mybir as mybir
import concourse.tile as tile

B = 256
D = 2048
S = 65536
TEMP = 0.05
N_CORES = 8
SHARD = S // N_CORES          # 8192 rows of the memory bank per core
JC = 512                      # j-chunk width (one PSUM bank of f32)
N_CHUNKS = SHARD // JC        # 16
KT = D // 128                 # 16 k-tiles of 128

# Matmul operand dtype mode: "f32" (exact, 4 cyc/row), "f32r" (full rate),
# "bf16" (half DMA traffic, inputs rounded on host), "fp8" (quarter DMA
# traffic + DoubleRow 2-MAC/cell, inputs scaled by FP8_SCALE then e4m3).
MODE = os.environ.get("CM_MODE", "fp8")

# e4m3 normal range starts at 2^-6; x/feats components are ~N(0, 1/2048)
# (sigma 0.022), so scale by 2^6 to keep ~99% of them normal.  The matmul
# then computes (64x)·(64f); the 1/4096 is folded into the ACT exp scale.
FP8_SCALE = 64.0


def build_nc(mode=MODE):
    f32 = mybir.dt.float32
    if mode == "bf16":
        in_dt = mybir.dt.bfloat16
    elif mode == "fp8":
        in_dt = mybir.dt.float8e4
    elif mode == "f32r":
        in_dt = mybir.dt.float32r
    else:
        in_dt = f32
    act_scale = 1.0 / TEMP
    if mode == "fp8":
        act_scale /= FP8_SCALE * FP8_SCALE

    nc = bacc.Bacc("TRN2", target_bir_lowering=False, debug=False,
                   num_devices=N_CORES)
    xT_d = nc.dram_tensor("xT", [128, KT, B], in_dt, kind="ExternalInput")
    featsT_d = nc.dram_tensor("featsT", [N_CHUNKS, 128, KT, JC], in_dt,
                              kind="ExternalInput")
    s_d = nc.dram_tensor("s_out", [128, 2], f32, kind="ExternalOutput")

    with tile.TileContext(nc) as tc:
        with (
            tc.tile_pool(name="xpool", bufs=1) as xpool,
            tc.tile_pool(name="fpool", bufs=6) as fpool,
            tc.tile_pool(name="spool", bufs=1) as spool,
            tc.tile_pool(name="jpool", bufs=4) as jpool,
            tc.tile_pool(name="psum", bufs=4, space="PSUM") as ppool,
        ):
            # xT gates the very first matmul -> load it first.  Chunk 0 is
            # split into k-pair sub-DMAs so the PE can start on the first
            # pair instead of waiting for the whole 1 MB chunk.
            # ACT's HWDGE ring, so it doesn't serialize ahead of the
            # feature stream on the SP ring.
            xT = xpool.tile([128, KT, B], in_dt)
            nc.scalar.dma_start(out=xT[:], in_=xT_d[:])

            f_tiles = {}
            f_tiles[0] = fpool.tile([128, KT, JC], in_dt, tag="feats",
                                    name="f_tile")
            for t in range(KT // 2):
                nc.sync.dma_start(out=f_tiles[0][:, 2 * t:2 * t + 2, :],
                                  in_=featsT_d[0, :, 2 * t:2 * t + 2, :])

            sums = spool.tile([128, 2, N_CHUNKS], f32)
            s_final = spool.tile([128, 2], f32)

            for jc in range(N_CHUNKS):
                if jc not in f_tiles:
                    f_tiles[jc] = fpool.tile([128, KT, JC], in_dt,
                                             tag="feats", name="f_tile")
                    nc.sync.dma_start(out=f_tiles[jc][:], in_=featsT_d[jc])
                f_tile = f_tiles[jc]
                for bh in range(2):
                    ps = ppool.tile([128, JC], f32, tag="ps")
                    bsl = slice(bh * 128, (bh + 1) * 128)
                    if mode == "fp8":
                        # DoubleRow: one matmul consumes two adjacent
                        # k-tiles; operands are [128, 2, dim] APs.
                        for t in range(KT // 2):
                            nc.tensor.matmul(
                                ps[:],
                                xT[:, 2 * t:2 * t + 2, bsl],
                                f_tile[:, 2 * t:2 * t + 2, :],
                                start=(t == 0), stop=(t == KT // 2 - 1),
                                perf_mode=mybir.MatmulPerfMode.DoubleRow)
                    else:
                        for kk in range(KT):
                            nc.tensor.matmul(
                                ps[:], xT[:, kk, bsl], f_tile[:, kk, :],
                                start=(kk == 0), stop=(kk == KT - 1))
                    junk = jpool.tile([128, JC], f32, tag="junk")
                    nc.scalar.activation(
                        junk[:], ps[:], mybir.ActivationFunctionType.Exp,
                        scale=act_scale,
                        accum_out=sums[:, bh, jc:jc + 1])

            for bh in range(2):
                nc.vector.reduce_sum(s_final[:, bh:bh + 1], sums[:, bh, :],
                                     axis=mybir.AxisListType.X)
            nc.sync.dma_start(out=s_d[:], in_=s_final[:])

    nc.compile()
    return nc


_NC_CACHE = {}


def _get_nc(mode=MODE):
    if mode not in _NC_CACHE:
        _NC_CACHE[mode] = build_nc(mode)
    return _NC_CACHE[mode]


def host_prep(inputs, features, mode=MODE):
    """Normalize/transpose/pack on host; returns (x_norm_f32, in_maps)."""
    x = np.asarray(inputs, dtype=np.float32)
    x = x / np.linalg.norm(x, axis=1, keepdims=True)
    scale = np.float32(1.0)
    if mode == "bf16":
        import ml_dtypes
        np_dt = ml_dtypes.bfloat16
    elif mode == "fp8":
        import ml_dtypes
        np_dt = ml_dtypes.float8_e4m3
        scale = np.float32(FP8_SCALE)
    else:
        np_dt = np.float32

    # xT[p, kk, b] = x[b, kk*128 + p]
    xT = np.ascontiguousarray(
        (x.T * scale).reshape(KT, 128, B).transpose(1, 0, 2).astype(np_dt))

    feats = np.asarray(features, dtype=np.float32)
    if mode == "fp8":
        feats = feats * scale
    in_maps = []
    for c in range(N_CORES):
        # shardT[k, j] = feats[c*SHARD + j, k]; packed[jc, p, kk, j] =
        # shardT[kk*128 + p, jc*JC + j]
        shardT = feats[c * SHARD:(c + 1) * SHARD].T       # [D, SHARD] view
        packed = np.ascontiguousarray(
            shardT.reshape(KT, 128, N_CHUNKS, JC).transpose(2, 1, 0, 3)
            .astype(np_dt))
        in_maps.append({"xT": xT, "featsT": packed})
    return x, in_maps


def combine(x, features, targets, core_outs):
    """Host combine: sum shard normalizers, add the target-logit term."""
    S_total = np.zeros(B, dtype=np.float64)
    for out in core_outs:
        s = out["s_out"].astype(np.float64)       # [128, 2]
        S_total += s.T.reshape(-1)                # item i = h*128 + p
    t = np.asarray(targets).astype(np.int64)
    f_t = np.asarray(features, dtype=np.float32)[t]          # [B, D]
    l_tgt = np.einsum("ij,ij->i", x.astype(np.float64),
                      f_t.astype(np.float64)) / TEMP
    loss = np.mean(np.log(S_total) - l_tgt)
    return np.array(loss, dtype=np.float32)


def kernel(**inputs):
    from concourse.bass_utils import run_bass_kernel_spmd

    x, in_maps = host_prep(inputs["inputs"], inputs["features"])
    nc = _get_nc()
    res = run_bass_kernel_spmd(nc, in_maps, list(range(N_CORES)))
    return combine(x, inputs["features"], inputs["targets"], res.results)
